# revision 23
# baseline (speedup 1.0000x reference)
"""KGramEmbeddingMLP on 8 TRN2 NeuronCores.

Model: one-hot context [256, 8*50257] -> embedding lookup -> MLP
512->1024->1024 (silu) -> vocab head 1024->50257.

The one-hot input is re-encoded host-side as indices (a lossless input
transform, like the baseline's host transpose); the device performs the
embedding lookup with SWDGE dma_gather (pair-packed rows to fit the int16
index field), so no 400MB one-hot ever crosses HBM.

Sharding: every core redundantly computes the full-batch embedding + MLP
(tiny: ~0.9 GFLOP) which removes the all-gather collective entirely; the
vocab head is tensor-parallel (each core owns 6288 of 50304 padded logit
columns, W3 column-chunked so TensorE can chase the DMA).

Queues: ALL loads ride the sync HWDGE ring in latency order — Tile
assigns the 8 DMAHW completion lanes round-robin across BOTH HWDGE rings
following its static schedule, so splitting loads/stores across the two
rings creates cross-ring lane-ordering stalls (observed: W3 chunk issues
waiting 28us on out-DMA lane ticks). Output stores go via SWDGE
(gpsimd): separate DMASW completion lanes, and the Q7 is idle during the
head. The scalar engine stays DMA-free so SILUs never queue behind
DMA-issue ring-credit backpressure (observed +5us). HAM keep-warm dummy
matmuls (start=False into the unused half of PSUM bank 7) extend each
gather's MM burst past the ~3.4us warm threshold.

Rejected after measurement: icache pre-fault via partition_broadcast
(the ~14us Q7 icache fill starts at first-instruction pickup no matter
what faults it); 4-way sharded gather + remote_dma_broadcast XOR
exchange (numerically correct, but PJRT launches the 8 per-device
executions ~2ms apart, so any cross-core wait inflates core 0's exec
time to ~14ms); fp8 head (e4m3 ~3.6% rel err vs the 2e-2 budget);
splitting the last gather into 2x256 + delaying W3 chunks 6-12 behind
the selects (sub-gathers add ~1.8us of Q7 desc-gen overhead, and the
gather-window HBM contention comes from W3 chunks 0-5, which cannot be
delayed without starving the head).

dtypes: table/W1/W2/W3/activations bf16, PSUM f32, logits stored bf16 and
upcast to f32 on host.
"""

import numpy as np
import ml_dtypes

VOCAB = 50257
K = 8
EMBED = 64
HIDDEN = 1024
BATCH = 256
NCORES = 8

NP = (VOCAB + 1) // 2   # 25129 pair-packed table rows
NS = BATCH * K          # 2048 gather slots (full batch, slot = k*256 + b)
NSG = 512               # slots per dma_gather (SWDGE ring holds 128 descs)
NG = NS // NSG          # 4 gathers
NWARM = 128             # warmup gather slots (absorbs Q7 icache miss)
IDXCOLS = NWARM // 16 + NS // 16  # 8 + 128

VP = 50304              # vocab padded to a multiple of 8*16
VS = VP // NCORES       # 6288 head columns per core
NCH = 13                # 12 x 512 + 1 x 144 column chunks
CHW = 512
LCHW = VS - 12 * CHW    # 144

KT1 = (K * EMBED) // 128   # 4 contraction tiles for W1
KT2 = HIDDEN // 128        # 8 contraction tiles for W2 / W3
MT = HIDDEN // 128         # 8 hidden output tiles

BF16 = ml_dtypes.bfloat16

TRACE = False           # test.py sets this to capture a neuron profile
LAST_RESULT = None      # BassKernelResults from the most recent run
_ACT = None             # sim_check overrides (CoreSim lacks Silu)

_compiled = {}


def _build():
    import concourse.mybir as mybir
    import concourse.tile as tile
    from concourse.tile_rust import add_dep_helper
    from concourse import bacc
    from concourse import library_config

    f32 = mybir.dt.float32
    bf16 = mybir.dt.bfloat16
    i16 = mybir.dt.int16

    nc = bacc.Bacc(
        "TRN2", target_bir_lowering=False, debug=False, num_devices=NCORES,
        num_swdge_queues=1,
    )
    act_fn = _ACT if _ACT is not None else mybir.ActivationFunctionType.Silu

    emb_d = nc.dram_tensor("embp", [NP, 128], bf16, kind="ExternalInput")
    idx_d = nc.dram_tensor("idxw", [128, IDXCOLS], i16, kind="ExternalInput")
    mk_d = nc.dram_tensor("mk", [128, NS], bf16, kind="ExternalInput")
    w1_d = nc.dram_tensor("w1", [K * EMBED, HIDDEN], bf16, kind="ExternalInput")
    b1_d = nc.dram_tensor("b1t", [128, MT], f32, kind="ExternalInput")
    w2_d = nc.dram_tensor("w2", [HIDDEN, HIDDEN], bf16, kind="ExternalInput")
    b2_d = nc.dram_tensor("b2t", [128, MT], f32, kind="ExternalInput")
    w3_d = nc.dram_tensor("w3b", [NCH - 1, 128, KT2 * CHW], bf16, kind="ExternalInput")
    w3l_d = nc.dram_tensor("w3l", [128, KT2 * LCHW], bf16, kind="ExternalInput")
    b3_d = nc.dram_tensor("b3b", [128, VS], bf16, kind="ExternalInput")
    out_d = nc.dram_tensor("out", [BATCH, VS], bf16, kind="ExternalOutput")

    with tile.TileContext(nc) as tc:
        with (
            tc.tile_pool(name="const", bufs=1) as const,
            tc.tile_pool(name="gath", bufs=1) as gath,
            tc.tile_pool(name="mlp", bufs=1) as mlp,
            tc.tile_pool(name="head", bufs=6) as head,
            tc.tile_pool(name="psum", bufs=8, space="PSUM") as psum,
        ):
            nc.gpsimd.load_library(library_config.mlp)

            # ---- all loads ride the sync ring, in latency order ---------
            # (One load ring: Tile assigns the 8 DMAHW completion lanes
            # round-robin across BOTH HWDGE rings following its static
            # schedule, so a second DMA ring creates cross-ring lane
            # ordering stalls — observed: W3 chunk issues waiting 28us on
            # out-DMA lane ticks. Stores go via SWDGE instead.)
            idx_sb = const.tile([128, IDXCOLS], i16, tag="idx")
            nc.sync.dma_start(idx_sb[:], idx_d[:])

            # The first gather (g0) absorbs the Q7 icache fill (~14us,
            # measured to start at first-instruction pickup regardless of
            # which extended-inst faults it) and queue init directly.
            # Desc-gens serialize on the Q7 FIFO (concurrent desc-gens on
            # different Q7 cores clobber each other's idx pointer:
            # observed gather q fetching gather q+1's rows), while the
            # SDMA rings still carry the transfers in parallel.

            # even/odd select masks: rows 0-63 = (1-parity), 64-127 = parity
            mk_sb = const.tile([128, NS], bf16, tag="mk")
            nc.sync.dma_start(mk_sb[:], mk_d[:])

            # Per-gather idx staging via vector copies: gpsimd dispatches
            # concurrently across its 8 Q7 cores, so each gather needs its
            # OWN non-elidable wait (distinct vector-engine ticks).
            idxg = []
            for q in range(NG):
                t = gath.tile([128, NSG // 16], i16, tag=f"idxg{q}")
                nc.vector.tensor_copy(
                    t[:],
                    idx_sb[:, NWARM // 16 + q * (NSG // 16):
                           NWARM // 16 + (q + 1) * (NSG // 16)],
                )
                idxg.append(t)

            # ---- biases + silu table prewarm (scalar ring) --------------
            b1_sb = const.tile([128, MT], f32, tag="b1")
            nc.sync.dma_start(b1_sb[:], b1_d[:])
            b2_sb = const.tile([128, MT], f32, tag="b2")
            nc.sync.dma_start(b2_sb[:], b2_d[:])
            actwarm = const.tile([128, 1], f32, tag="actwarm")
            nc.scalar.activation(
                actwarm[:], b1_sb[:, 0:1], act_fn, bias=b1_sb[:, 0:1]
            )
            # ---- bulk weight stream on the sync ring --------------------
            # Early batch: W3 chunks 0-7 (head's first group needs them by
            # ~54us) + W1 (phase 1, ~40us). Deferred batch (w2/b3b/W3
            # chunks 8-12, all with >=12us of slack) is held behind g2's
            # select so the ring is near-idle while the LAST gather's
            # SWDGE transfer lands (observed ~4.8us exposed when contended).
            w3c = []
            w3_dmas = []
            for ch in range(NCH):
                w = CHW if ch < NCH - 1 else LCHW
                t = const.tile([128, KT2 * w], bf16, tag=f"w3c_{ch}")
                w3_dmas.append(nc.sync.dma_start(
                    t[:], w3_d[ch] if ch < NCH - 1 else w3l_d[:]))
                w3c.append(t)
            w1_sb = []
            for kk in range(KT1):
                t = const.tile([128, HIDDEN], bf16, tag=f"w1_{kk}")
                nc.sync.dma_start(t[:], w1_d[kk * 128:(kk + 1) * 128, :])
                w1_sb.append(t)
            w2_sb = []
            w2_dmas = []
            for kk in range(KT2):
                t = const.tile([128, HIDDEN], bf16, tag=f"w2_{kk}")
                w2_dmas.append(nc.sync.dma_start(
                    t[:], w2_d[kk * 128:(kk + 1) * 128, :]))
                w2_sb.append(t)
            b3b_sb = const.tile([128, VS], bf16, tag="b3b")
            b3b_dma = nc.sync.dma_start(b3b_sb[:], b3_d[:])

            # ---- gather + select straight into xt tiles ---------------
            # gather q covers slots [512q, 512q+512) = k=2q (batch 0:256)
            # then k=2q+1; xt tile q partitions (k%2)*64+e.
            xt = []
            for t_i in range(KT1):
                t = mlp.tile([128, BATCH], bf16, tag=f"xt_{t_i}")
                xt.append(t)
            h1p = []
            for m in range(MT):
                t = psum.tile([128, 2 * BATCH], f32, tag="ps")
                h1p.append(t)
            sA = gath.tile([64, NSG], bf16, tag="sA")
            sB = gath.tile([64, NSG], bf16, tag="sB")
            g_tiles = []
            for q in range(NG):
                gt = gath.tile([128, 1, NSG], bf16, tag=f"g{q}")
                g_tiles.append(gt)
            for q in range(NG):
                g = g_tiles[q]
                nc.gpsimd.dma_gather(
                    g[:], emb_d[:], idxg[q][:],
                    NSG, NSG, 128, transpose=True, queue_num=0,
                )
                s = slice(q * NSG, (q + 1) * NSG)
                nc.vector.tensor_mul(sA[:], g[0:64, 0, :], mk_sb[0:64, s])
                nc.vector.tensor_mul(sB[:], g[64:128, 0, :], mk_sb[64:128, s])
                nc.vector.tensor_add(xt[q][0:64, :], sA[:, 0:BATCH], sB[:, 0:BATCH])
                a2 = nc.vector.tensor_add(
                    xt[q][64:128, :], sA[:, BATCH:], sB[:, BATCH:]
                )
                if q == 2:
                    for d in (*w2_dmas, b3b_dma, *w3_dmas[8:]):
                        add_dep_helper(
                            d.ins, a2.ins,
                            reason="hold deferred loads off g3's transfer",
                        )
                # h1 partial accumulation for contraction tile kk=q
                for m in range(MT):
                    nc.tensor.matmul(
                        h1p[m][:, :BATCH],
                        w1_sb[q][:, m * 128:(m + 1) * 128],
                        xt[q][:],
                        start=(q == 0),
                        stop=(q == NG - 1),
                    )
                # HAM keep-warm: garbage matmuls into the unused half of
                # bank 7 (start=False so the live accumulation's
                # has_written bits are untouched), chained on THIS
                # gather's raw tile so they extend the real MM burst past
                # the ~3.4us HAM warm threshold with no idle gap.
                if q < NG - 1:
                    for _d in range(8):
                        nc.tensor.matmul(
                            h1p[7][:, BATCH:2 * BATCH],
                            w1_sb[0][:, 0:128],
                            g_tiles[q][:, 0, 0:BATCH],
                            start=False,
                            stop=True,
                        )

            # ---- phase 2: silu -> h2 ----------------------------------
            h1t = []
            for m in range(MT):
                t = mlp.tile([128, BATCH], bf16, tag=f"h1_{m}")
                nc.scalar.activation(
                    t[:], h1p[m][:, :BATCH],
                    act_fn,
                    bias=b1_sb[:, m:m + 1],
                )
                h1t.append(t)

            h2t = []
            for m in range(MT):
                ps = psum.tile([128, 2 * BATCH], f32, tag="ps")
                for kk in range(KT2):
                    nc.tensor.matmul(
                        ps[:, :BATCH],
                        w2_sb[kk][:, m * 128:(m + 1) * 128],
                        h1t[kk][:],
                        start=(kk == 0),
                        stop=(kk == KT2 - 1),
                    )
                t = mlp.tile([128, BATCH], bf16, tag=f"h2_{m}")
                nc.scalar.activation(
                    t[:], ps[:, :BATCH],
                    act_fn,
                    bias=b2_sb[:, m:m + 1],
                )
                h2t.append(t)

            # ---- phase 3: logits[:, shard] = h2 @ W3s + b3s -----------
            # r=1 ends with a small group so the tail (serial DVE bias
            # adds + last out DMA) is short.
            groups_r = [
                [list(range(8)), list(range(8, 13))],
                [list(range(5)), list(range(5, 10)), list(range(10, 13))],
            ]
            for r in range(BATCH // 128):
                for grp in groups_r[r]:
                    pss = {}
                    for ch in grp:
                        t = psum.tile([128, CHW], f32, tag="ps")
                        pss[ch] = t
                    for kk in range(KT2):
                        for ch in grp:
                            w = CHW if ch < NCH - 1 else LCHW
                            nc.tensor.matmul(
                                pss[ch][:, :w],
                                h2t[kk][:, r * 128:(r + 1) * 128],
                                w3c[ch][:, kk * w:(kk + 1) * w],
                                start=(kk == 0),
                                stop=(kk == KT2 - 1),
                            )
                    if r == 1 and grp[0] == 10:
                        # tail group: stage all three chunks into one
                        # contiguous tile -> single store DMA
                        osb = head.tile([128, 2 * CHW + LCHW], bf16,
                                        tag="osbt")
                        for i, ch in enumerate(grp):
                            w = CHW if ch < NCH - 1 else LCHW
                            nc.vector.tensor_add(
                                osb[:, i * CHW:i * CHW + w],
                                pss[ch][:, :w],
                                b3b_sb[:, ch * CHW:ch * CHW + w],
                            )
                        nc.gpsimd.dma_start(
                            out_d[128:256, 10 * CHW:VS], osb[:]
                        )
                    else:
                        for ch in grp:
                            w = CHW if ch < NCH - 1 else LCHW
                            off = ch * CHW
                            osb = head.tile([128, CHW], bf16, tag="osb")
                            nc.vector.tensor_add(
                                osb[:, :w], pss[ch][:, :w], b3b_sb[:, off:off + w]
                            )
                            nc.gpsimd.dma_start(
                                out_d[r * 128:(r + 1) * 128, off:off + w],
                                osb[:, :w],
                            )

    nc.compile()
    return nc


def _get_nc():
    if "nc" not in _compiled:
        _compiled["nc"] = _build()
    return _compiled["nc"]


def _prep_inputs(context_flat, embed_w, W1, b1, W2, b2, W3, b3):
    ctx = np.asarray(context_flat).reshape(BATCH, K, VOCAB)
    idx = np.argmax(ctx, axis=-1)                    # [B, K]
    idx_flat = np.ascontiguousarray(idx.T).reshape(-1)  # slot = k*256 + b

    # warmup cols (zeros) + pair indices wrapped [i%16, i//16], replicated
    idx2 = (idx_flat >> 1).astype(np.int16)
    idx_w = np.zeros((16, IDXCOLS), np.int16)
    idx_w[:, NWARM // 16:] = idx2.reshape(NS // 16, 16).T
    idx_w = np.tile(idx_w, (8, 1))

    m1 = (idx_flat & 1).astype(np.float32)
    mk = np.empty((128, NS), np.float32)
    mk[0:64, :] = (1.0 - m1)[None, :]
    mk[64:128, :] = m1[None, :]
    mk = mk.astype(BF16)

    emb_b = np.asarray(embed_w, np.float32).astype(BF16)
    embp = np.zeros((NP, 128), BF16)
    embp[:, 0:64] = emb_b[0::2][:NP]
    odd = emb_b[1::2]
    embp[:odd.shape[0], 64:128] = odd

    w1 = np.asarray(W1, np.float32).astype(BF16)
    w2 = np.asarray(W2, np.float32).astype(BF16)
    b1t = np.ascontiguousarray(np.asarray(b1, np.float32).reshape(MT, 128).T)
    b2t = np.ascontiguousarray(np.asarray(b2, np.float32).reshape(MT, 128).T)

    w3_p = np.zeros((HIDDEN, VP), BF16)
    w3_p[:, :VOCAB] = np.asarray(W3, np.float32).astype(BF16)
    b3_p = np.zeros((1, VP), BF16)
    b3_p[0, :VOCAB] = np.asarray(b3, np.float32).astype(BF16)

    in_maps = []
    for c in range(NCORES):
        shard = w3_p[:, c * VS:(c + 1) * VS]          # [1024, 6288]
        # chunk ch as [128, kk*w+n]: one DMA = 128 contiguous 8KB lines
        main = shard[:, :12 * CHW].reshape(KT2, 128, 12, CHW)
        w3b = np.ascontiguousarray(
            main.transpose(2, 1, 0, 3)).reshape(12, 128, KT2 * CHW)
        w3l = np.ascontiguousarray(
            shard[:, 12 * CHW:].reshape(KT2, 128, LCHW)
            .transpose(1, 0, 2)).reshape(128, KT2 * LCHW)
        in_maps.append({
            "embp": embp,
            "idxw": idx_w,
            "mk": mk,
            "w1": w1,
            "b1t": b1t,
            "w2": w2,
            "b2t": b2t,
            "w3b": w3b,
            "w3l": w3l,
            "b3b": np.ascontiguousarray(np.broadcast_to(
                b3_p[:, c * VS:(c + 1) * VS], (128, VS))),
        })
    return in_maps


def kernel(**inputs):
    global LAST_RESULT
    from concourse import bass_utils

    nc = _get_nc()
    in_maps = _prep_inputs(**inputs)
    res = bass_utils.run_bass_kernel_spmd(
        nc, in_maps, core_ids=list(range(NCORES)), trace=TRACE
    )
    LAST_RESULT = res
    full = np.empty((BATCH, VP), np.float32)
    for c in range(NCORES):
        full[:, c * VS:(c + 1) * VS] = res.results[c]["out"].astype(np.float32)
    return np.ascontiguousarray(full[:, :VOCAB])


# revision 25
# speedup vs baseline: 1.0212x; 1.0212x over previous
"""KGramEmbeddingMLP on 8 TRN2 NeuronCores.

Model: one-hot context [256, 8*50257] -> embedding lookup -> MLP
512->1024->1024 (silu) -> vocab head 1024->50257.

The one-hot input is re-encoded host-side as indices (a lossless input
transform, like the baseline's host transpose); the device performs the
embedding lookup with SWDGE dma_gather (pair-packed rows to fit the int16
index field), so no 400MB one-hot ever crosses HBM.

Sharding: every core redundantly computes the full-batch embedding + MLP
(tiny: ~0.9 GFLOP) which removes the all-gather collective entirely; the
vocab head is tensor-parallel (each core owns 6288 of 50304 padded logit
columns, W3 column-chunked so TensorE can chase the DMA).

Queue split (v2): small latency-critical inputs (idx, masks, biases) and
the output stores ride the SCALAR HWDGE ring; the bulk weight stream
(W1/W2/W3) rides the SYNC ring. This keeps the SILU activations (scalar
engine) from queueing behind 13 x 1MB W3 DMA issues (observed +5us), and
gets the gather's index tile on-chip by ~6us so the Q7 icache-miss warmup
gather dispatches immediately.

dtypes: table/W1/W2/W3/activations bf16, PSUM f32, logits stored bf16 and
upcast to f32 on host.
"""

import numpy as np
import ml_dtypes

VOCAB = 50257
K = 8
EMBED = 64
HIDDEN = 1024
BATCH = 256
NCORES = 8

NP = (VOCAB + 1) // 2   # 25129 pair-packed table rows
NS = BATCH * K          # 2048 gather slots (full batch, slot = k*256 + b)
NSG = 512               # slots per dma_gather (SWDGE ring holds 128 descs)
NG = NS // NSG          # 4 gathers
NWARM = 128             # warmup gather slots (absorbs Q7 icache miss)
IDXCOLS = NWARM // 16 + NS // 16  # 8 + 128

VP = 50304              # vocab padded to a multiple of 8*16
VS = VP // NCORES       # 6288 head columns per core
NCH = 13                # 12 x 512 + 1 x 144 column chunks
CHW = 512
LCHW = VS - 12 * CHW    # 144

KT1 = (K * EMBED) // 128   # 4 contraction tiles for W1
KT2 = HIDDEN // 128        # 8 contraction tiles for W2 / W3
MT = HIDDEN // 128         # 8 hidden output tiles

BF16 = ml_dtypes.bfloat16

TRACE = False           # test.py sets this to capture a neuron profile
LAST_RESULT = None      # BassKernelResults from the most recent run
_ACT = None             # sim_check overrides (CoreSim lacks Silu)

_compiled = {}


def _build():
    import concourse.mybir as mybir
    import concourse.tile as tile
    from concourse import bacc
    from concourse import library_config

    f32 = mybir.dt.float32
    bf16 = mybir.dt.bfloat16
    i16 = mybir.dt.int16

    nc = bacc.Bacc(
        "TRN2", target_bir_lowering=False, debug=False, num_devices=NCORES,
        num_swdge_queues=1,
    )
    act_fn = _ACT if _ACT is not None else mybir.ActivationFunctionType.Silu

    emb_d = nc.dram_tensor("embp", [NP, 128], bf16, kind="ExternalInput")
    idx_d = nc.dram_tensor("idxw", [128, IDXCOLS], i16, kind="ExternalInput")
    mk_d = nc.dram_tensor("mk", [128, NS], bf16, kind="ExternalInput")
    w1_d = nc.dram_tensor("w1", [K * EMBED, HIDDEN], bf16, kind="ExternalInput")
    b1_d = nc.dram_tensor("b1t", [128, MT], f32, kind="ExternalInput")
    w2_d = nc.dram_tensor("w2", [HIDDEN, HIDDEN], bf16, kind="ExternalInput")
    b2_d = nc.dram_tensor("b2t", [128, MT], f32, kind="ExternalInput")
    w3_d = nc.dram_tensor("w3b", [NCH - 1, 128, KT2 * CHW], bf16, kind="ExternalInput")
    w3l_d = nc.dram_tensor("w3l", [128, KT2 * LCHW], bf16, kind="ExternalInput")
    b3_d = nc.dram_tensor("b3b", [128, VS], bf16, kind="ExternalInput")
    out_d = nc.dram_tensor("out", [BATCH, VS], bf16, kind="ExternalOutput")

    with tile.TileContext(nc) as tc:
        with (
            tc.tile_pool(name="const", bufs=1) as const,
            tc.tile_pool(name="gath", bufs=1) as gath,
            tc.tile_pool(name="mlp", bufs=1) as mlp,
            tc.tile_pool(name="head", bufs=6) as head,
            tc.tile_pool(name="psum", bufs=8, space="PSUM") as psum,
        ):
            nc.gpsimd.load_library(library_config.mlp)

            # ---- all loads ride the sync ring, in latency order ---------
            # (One load ring: Tile assigns the 8 DMAHW completion lanes
            # round-robin across BOTH HWDGE rings following its static
            # schedule, so a second DMA ring creates cross-ring lane
            # ordering stalls — observed: W3 chunk issues waiting 28us on
            # out-DMA lane ticks. Stores go via SWDGE instead.)
            idx_sb = const.tile([128, IDXCOLS], i16, tag="idx")
            nc.sync.dma_start(idx_sb[:], idx_d[:])

            # The first gather (g0) absorbs the Q7 icache fill (~14us,
            # measured to start at first-instruction pickup regardless of
            # which extended-inst faults it) and queue init directly.
            # Desc-gens serialize on the Q7 FIFO (concurrent desc-gens on
            # different Q7 cores clobber each other's idx pointer:
            # observed gather q fetching gather q+1's rows), while the
            # SDMA rings still carry the transfers in parallel.

            # even/odd select masks: rows 0-63 = (1-parity), 64-127 = parity
            mk_sb = const.tile([128, NS], bf16, tag="mk")
            nc.sync.dma_start(mk_sb[:], mk_d[:])

            # Per-gather idx staging via vector copies: gpsimd dispatches
            # concurrently across its 8 Q7 cores, so each gather needs its
            # OWN non-elidable wait (distinct vector-engine ticks).
            idxg = []
            for q in range(NG):
                t = gath.tile([128, NSG // 16], i16, tag=f"idxg{q}")
                nc.vector.tensor_copy(
                    t[:],
                    idx_sb[:, NWARM // 16 + q * (NSG // 16):
                           NWARM // 16 + (q + 1) * (NSG // 16)],
                )
                idxg.append(t)

            # ---- biases + silu table prewarm (scalar ring) --------------
            b1_sb = const.tile([128, MT], f32, tag="b1")
            nc.sync.dma_start(b1_sb[:], b1_d[:])
            b2_sb = const.tile([128, MT], f32, tag="b2")
            nc.sync.dma_start(b2_sb[:], b2_d[:])
            actwarm = const.tile([128, 1], f32, tag="actwarm")
            nc.scalar.activation(
                actwarm[:], b1_sb[:, 0:1], act_fn, bias=b1_sb[:, 0:1]
            )
            b3b_sb = const.tile([128, VS], bf16, tag="b3b")
            nc.sync.dma_start(b3b_sb[:], b3_d[:])

            # ---- bulk weight stream on the sync ring --------------------
            w1_sb = []
            for kk in range(KT1):
                t = const.tile([128, HIDDEN], bf16, tag=f"w1_{kk}")
                nc.sync.dma_start(t[:], w1_d[kk * 128:(kk + 1) * 128, :])
                w1_sb.append(t)
            w2_sb = []
            for kk in range(KT2):
                t = const.tile([128, HIDDEN], bf16, tag=f"w2_{kk}")
                nc.sync.dma_start(t[:], w2_d[kk * 128:(kk + 1) * 128, :])
                w2_sb.append(t)
            # W3 column chunks, in compute order (host has pre-blocked each
            # chunk so a DMA is 128 contiguous 8KB lines)
            w3c = []
            for ch in range(NCH):
                w = CHW if ch < NCH - 1 else LCHW
                t = const.tile([128, KT2 * w], bf16, tag=f"w3c_{ch}")
                nc.sync.dma_start(t[:], w3_d[ch] if ch < NCH - 1 else w3l_d[:])
                w3c.append(t)

            # ---- gather + select straight into xt tiles ---------------
            # gather q covers slots [512q, 512q+512) = k=2q (batch 0:256)
            # then k=2q+1; xt tile q partitions (k%2)*64+e.
            xt = []
            for t_i in range(KT1):
                t = mlp.tile([128, BATCH], bf16, tag=f"xt_{t_i}")
                xt.append(t)
            h1p = []
            for m in range(MT):
                t = psum.tile([128, 2 * BATCH], f32, tag="ps")
                h1p.append(t)
            sA = gath.tile([64, NSG], bf16, tag="sA")
            sB = gath.tile([64, NSG], bf16, tag="sB")
            g_tiles = []
            for q in range(NG):
                gt = gath.tile([128, 1, NSG], bf16, tag=f"g{q}")
                g_tiles.append(gt)
            for q in range(NG):
                g = g_tiles[q]
                nc.gpsimd.dma_gather(
                    g[:], emb_d[:], idxg[q][:],
                    NSG, NSG, 128, transpose=True, queue_num=0,
                )
                s = slice(q * NSG, (q + 1) * NSG)
                nc.vector.tensor_mul(sA[:], g[0:64, 0, :], mk_sb[0:64, s])
                nc.vector.tensor_mul(sB[:], g[64:128, 0, :], mk_sb[64:128, s])
                nc.vector.tensor_add(xt[q][0:64, :], sA[:, 0:BATCH], sB[:, 0:BATCH])
                nc.vector.tensor_add(
                    xt[q][64:128, :], sA[:, BATCH:], sB[:, BATCH:]
                )
                # h1 partial accumulation for contraction tile kk=q
                for m in range(MT):
                    nc.tensor.matmul(
                        h1p[m][:, :BATCH],
                        w1_sb[q][:, m * 128:(m + 1) * 128],
                        xt[q][:],
                        start=(q == 0),
                        stop=(q == NG - 1),
                    )
                # HAM keep-warm: garbage matmuls into the unused half of
                # bank 7 (start=False so the live accumulation's
                # has_written bits are untouched), chained on THIS
                # gather's raw tile so they extend the real MM burst past
                # the ~3.4us HAM warm threshold with no idle gap.
                if q < NG - 1:
                    for _d in range(8):
                        nc.tensor.matmul(
                            h1p[7][:, BATCH:2 * BATCH],
                            w1_sb[0][:, 0:128],
                            g_tiles[q][:, 0, 0:BATCH],
                            start=False,
                            stop=True,
                        )

            # ---- phase 2: silu -> h2 ----------------------------------
            h1t = []
            for m in range(MT):
                t = mlp.tile([128, BATCH], bf16, tag=f"h1_{m}")
                nc.scalar.activation(
                    t[:], h1p[m][:, :BATCH],
                    act_fn,
                    bias=b1_sb[:, m:m + 1],
                )
                h1t.append(t)

            h2t = []
            for m in range(MT):
                ps = psum.tile([128, 2 * BATCH], f32, tag="ps")
                for kk in range(KT2):
                    nc.tensor.matmul(
                        ps[:, :BATCH],
                        w2_sb[kk][:, m * 128:(m + 1) * 128],
                        h1t[kk][:],
                        start=(kk == 0),
                        stop=(kk == KT2 - 1),
                    )
                t = mlp.tile([128, BATCH], bf16, tag=f"h2_{m}")
                nc.scalar.activation(
                    t[:], ps[:, :BATCH],
                    act_fn,
                    bias=b2_sb[:, m:m + 1],
                )
                h2t.append(t)

            # ---- phase 3: logits[:, shard] = h2 @ W3s + b3s -----------
            # r=1 ends with a small group so the tail (serial DVE bias
            # adds + last out DMA) is short.
            groups_r = [
                [list(range(8)), list(range(8, 13))],
                [list(range(5)), list(range(5, 10)), list(range(10, 13))],
            ]
            for r in range(BATCH // 128):
                for grp in groups_r[r]:
                    pss = {}
                    for ch in grp:
                        t = psum.tile([128, CHW], f32, tag="ps")
                        pss[ch] = t
                    for kk in range(KT2):
                        for ch in grp:
                            w = CHW if ch < NCH - 1 else LCHW
                            nc.tensor.matmul(
                                pss[ch][:, :w],
                                h2t[kk][:, r * 128:(r + 1) * 128],
                                w3c[ch][:, kk * w:(kk + 1) * w],
                                start=(kk == 0),
                                stop=(kk == KT2 - 1),
                            )
                    if r == 1 and grp[0] == 10:
                        # tail group: stage all three chunks into one
                        # contiguous tile -> single store DMA
                        osb = head.tile([128, 2 * CHW + LCHW], bf16,
                                        tag="osbt")
                        for i, ch in enumerate(grp):
                            w = CHW if ch < NCH - 1 else LCHW
                            nc.vector.tensor_add(
                                osb[:, i * CHW:i * CHW + w],
                                pss[ch][:, :w],
                                b3b_sb[:, ch * CHW:ch * CHW + w],
                            )
                        nc.gpsimd.dma_start(
                            out_d[128:256, 10 * CHW:VS], osb[:]
                        )
                    else:
                        for ch in grp:
                            w = CHW if ch < NCH - 1 else LCHW
                            off = ch * CHW
                            osb = head.tile([128, CHW], bf16, tag="osb")
                            nc.vector.tensor_add(
                                osb[:, :w], pss[ch][:, :w], b3b_sb[:, off:off + w]
                            )
                            nc.gpsimd.dma_start(
                                out_d[r * 128:(r + 1) * 128, off:off + w],
                                osb[:, :w],
                            )

    nc.compile()
    return nc


def _get_nc():
    if "nc" not in _compiled:
        _compiled["nc"] = _build()
    return _compiled["nc"]


def _prep_inputs(context_flat, embed_w, W1, b1, W2, b2, W3, b3):
    ctx = np.asarray(context_flat).reshape(BATCH, K, VOCAB)
    idx = np.argmax(ctx, axis=-1)                    # [B, K]
    idx_flat = np.ascontiguousarray(idx.T).reshape(-1)  # slot = k*256 + b

    # warmup cols (zeros) + pair indices wrapped [i%16, i//16], replicated
    idx2 = (idx_flat >> 1).astype(np.int16)
    idx_w = np.zeros((16, IDXCOLS), np.int16)
    idx_w[:, NWARM // 16:] = idx2.reshape(NS // 16, 16).T
    idx_w = np.tile(idx_w, (8, 1))

    m1 = (idx_flat & 1).astype(np.float32)
    mk = np.empty((128, NS), np.float32)
    mk[0:64, :] = (1.0 - m1)[None, :]
    mk[64:128, :] = m1[None, :]
    mk = mk.astype(BF16)

    emb_b = np.asarray(embed_w, np.float32).astype(BF16)
    embp = np.zeros((NP, 128), BF16)
    embp[:, 0:64] = emb_b[0::2][:NP]
    odd = emb_b[1::2]
    embp[:odd.shape[0], 64:128] = odd

    w1 = np.asarray(W1, np.float32).astype(BF16)
    w2 = np.asarray(W2, np.float32).astype(BF16)
    b1t = np.ascontiguousarray(np.asarray(b1, np.float32).reshape(MT, 128).T)
    b2t = np.ascontiguousarray(np.asarray(b2, np.float32).reshape(MT, 128).T)

    w3_p = np.zeros((HIDDEN, VP), BF16)
    w3_p[:, :VOCAB] = np.asarray(W3, np.float32).astype(BF16)
    b3_p = np.zeros((1, VP), BF16)
    b3_p[0, :VOCAB] = np.asarray(b3, np.float32).astype(BF16)

    in_maps = []
    for c in range(NCORES):
        shard = w3_p[:, c * VS:(c + 1) * VS]          # [1024, 6288]
        # chunk ch as [128, kk*w+n]: one DMA = 128 contiguous 8KB lines
        main = shard[:, :12 * CHW].reshape(KT2, 128, 12, CHW)
        w3b = np.ascontiguousarray(
            main.transpose(2, 1, 0, 3)).reshape(12, 128, KT2 * CHW)
        w3l = np.ascontiguousarray(
            shard[:, 12 * CHW:].reshape(KT2, 128, LCHW)
            .transpose(1, 0, 2)).reshape(128, KT2 * LCHW)
        in_maps.append({
            "embp": embp,
            "idxw": idx_w,
            "mk": mk,
            "w1": w1,
            "b1t": b1t,
            "w2": w2,
            "b2t": b2t,
            "w3b": w3b,
            "w3l": w3l,
            "b3b": np.ascontiguousarray(np.broadcast_to(
                b3_p[:, c * VS:(c + 1) * VS], (128, VS))),
        })
    return in_maps


def kernel(**inputs):
    global LAST_RESULT
    from concourse import bass_utils

    nc = _get_nc()
    in_maps = _prep_inputs(**inputs)
    res = bass_utils.run_bass_kernel_spmd(
        nc, in_maps, core_ids=list(range(NCORES)), trace=TRACE
    )
    LAST_RESULT = res
    full = np.empty((BATCH, VP), np.float32)
    for c in range(NCORES):
        full[:, c * VS:(c + 1) * VS] = res.results[c]["out"].astype(np.float32)
    return np.ascontiguousarray(full[:, :VOCAB])
